# revision 1
# baseline (speedup 1.0000x reference)
"""Trainium2 Bass kernel for a dense transformer block (nn_Block_7713761264306).

Sharding: 8 cores = 4 batches x 2 query-halves. Each core computes K/V over the
full sequence for its batch, but runs only its 1024 query rows through
attention and the FFN. The query half is selected by rotating the token axis
host-side (exact: no mask, softmax is permutation-invariant over keys).
No collectives.

Device layout: activations are kept feature-on-partition ([D, tokens]) so every
linear layer is a direct PE matmul (lhsT = weights, rhs = activations^T) with
no on-device transposes. LayerNorm statistics are computed with ones-vector
matmuls on the tensor engine; [1,N] row -> [128,N] broadcasts use K=1 matmuls.
Softmax runs in S^T layout (keys on partitions, queries free); denominators
come from a ones-column appended to V in the PV matmul. All matmuls are bf16
with fp32 PSUM accumulation; LN1's gamma/beta are folded into W_ap host-side.
o and h round-trip through DRAM so SBUF pool lifetimes nest (LIFO).
"""

import numpy as np
import ml_dtypes

import concourse.bass as bass
import concourse.mybir as mybir
import concourse.tile as tile
from concourse.bass import ts
from concourse.bass_utils import run_bass_kernel_spmd

BF16 = mybir.dt.bfloat16
F32 = mybir.dt.float32
bf16 = ml_dtypes.bfloat16

B, T, D, H, HS, FF = 4, 2048, 1024, 16, 64, 4096
P = 128
DC = D // P          # 8 feature chunks
FC = FF // P         # 32 ffn chunks
TKC = T // P         # 16 key/token chunks
Tq = 1024            # queries per core
NT = T // 512        # 4 column tiles over full seq
NQ = Tq // 512       # 2 column tiles over queries
N_CORES = 8
EPS = 1e-5

AF = mybir.ActivationFunctionType
ALU = mybir.AluOpType


def build_nc(cap=True):
    nc = bass.Bass()
    io = {}
    io["xt"] = nc.dram_tensor("xt", [DC, P, T], BF16, kind="ExternalInput")
    io["wq"] = nc.dram_tensor("wq", [DC, P, DC, P], BF16, kind="ExternalInput")
    io["wk"] = nc.dram_tensor("wk", [DC, P, DC, P], BF16, kind="ExternalInput")
    io["wv"] = nc.dram_tensor("wv", [DC, P, D], BF16, kind="ExternalInput")
    io["bqkv"] = nc.dram_tensor("bqkv", [P, 2 * DC], F32, kind="ExternalInput")
    io["bv"] = nc.dram_tensor("bv", [D], F32, kind="ExternalInput")
    io["wproj"] = nc.dram_tensor("wproj", [DC, P, DC, P], BF16,
                                 kind="ExternalInput")
    io["bproj"] = nc.dram_tensor("bproj", [P, DC], F32, kind="ExternalInput")
    io["w1"] = nc.dram_tensor("w1", [FC, P, DC, P], BF16, kind="ExternalInput")
    io["b1"] = nc.dram_tensor("b1", [P, FC], F32, kind="ExternalInput")
    io["w2"] = nc.dram_tensor("w2", [DC, P, FC, P], BF16, kind="ExternalInput")
    io["b2"] = nc.dram_tensor("b2", [P, DC], F32, kind="ExternalInput")
    io["g2"] = nc.dram_tensor("g2", [P, DC], F32, kind="ExternalInput")
    io["bln2"] = nc.dram_tensor("bln2", [P, DC], F32, kind="ExternalInput")
    io["out"] = nc.dram_tensor("out", [DC, P, Tq], F32, kind="ExternalOutput")

    with tile.TileContext(nc) as tc:
        _emit(nc, tc, io)
    nc.finalize()
    if cap:
        _cap_waits(nc)
    return nc


def _cap_waits(nc, keep_types=()):
    """This toolchain's walrus accepts only one sync-wait command per compute
    instruction; hoist extra waits into preceding same-engine NoOps."""
    cnt = 0
    for fn in nc.m.functions:
        for blk in fn.blocks:
            new = []
            for inst in blk.instructions:
                si = getattr(inst, "sync_info", None)
                if si is not None and len(si.on_wait) > 1 \
                        and type(inst).__name__ not in keep_types:
                    waits = list(si.on_wait)
                    for w in waits[:-1]:
                        cnt += 1
                        nop = mybir.InstNoOp(
                            name=f"{inst.name}-w{cnt}", ins=[], outs=[])
                        nop.engine = inst.engine
                        nop.sync_info = mybir.SyncInfo(on_wait=[w],
                                                       on_update=[])
                        new.append(nop)
                    inst.sync_info = mybir.SyncInfo(
                        on_wait=[waits[-1]], on_update=list(si.on_update))
                new.append(inst)
            blk.instructions = new
    return cnt


def _emit(nc, tc, io):
    xT_d = io["xt"]

    consts = tc.alloc_tile_pool(name="consts", bufs=1)

    # ------------------------- constants -------------------------
    bqkv_s = consts.tile([P, 2 * DC], F32)
    nc.sync.dma_start(out=bqkv_s, in_=io["bqkv"][:])
    bproj_s = consts.tile([P, DC], F32)
    nc.sync.dma_start(out=bproj_s, in_=io["bproj"][:])
    b1_s = consts.tile([P, FC], F32)
    nc.sync.dma_start(out=b1_s, in_=io["b1"][:])
    b2_s = consts.tile([P, DC], F32)
    nc.sync.dma_start(out=b2_s, in_=io["b2"][:])
    g2_s = consts.tile([P, DC], F32)
    nc.sync.dma_start(out=g2_s, in_=io["g2"][:])
    bln2_s = consts.tile([P, DC], F32)
    nc.sync.dma_start(out=bln2_s, in_=io["bln2"][:])
    bvB = consts.tile([P, D], F32)
    nc.sync.dma_start(out=bvB, in_=io["bv"][:].partition_broadcast(P))

    invD = consts.tile([P, 1], BF16)
    nc.vector.memset(invD, 1.0 / D)
    onesK1 = consts.tile([1, P], BF16)
    nc.vector.memset(onesK1, 1.0)
    eps_t = consts.tile([1, 1], F32)
    nc.vector.memset(eps_t, EPS)

    def ln_stats_bcast(pp_stat, pp_b, rpool, src_bf, src_sq, ncols, rB, murB,
                       tag):
        """Per-512 col block: mean/E[x^2] over partitions via PE ones-matmul,
        row math, then broadcast 1/sd and mu/sd to [P, 512] via K=1 matmul."""
        for kt in range(ncols // 512):
            ps_mu = pp_stat.tile([1, 512], F32, tag="st",
                                 name=f"psmu{tag}{kt}")
            for c in range(DC):
                nc.tensor.matmul(ps_mu, invD, src_bf[:, c, ts(kt, 512)],
                                 start=(c == 0), stop=(c == DC - 1))
            ps_sq = pp_stat.tile([1, 512], F32, tag="st",
                                 name=f"pssq{tag}{kt}")
            for c in range(DC):
                nc.tensor.matmul(ps_sq, invD, src_sq[:, c, ts(kt, 512)],
                                 start=(c == 0), stop=(c == DC - 1))
            mu = rpool.tile([1, 512], F32, tag="rowf", name=f"mu{tag}{kt}")
            nc.vector.tensor_copy(out=mu, in_=ps_mu)
            var = rpool.tile([1, 512], F32, tag="rowf", name=f"var{tag}{kt}")
            nc.vector.tensor_mul(var, mu, mu)
            nc.vector.tensor_sub(var, ps_sq, var)
            sd = rpool.tile([1, 512], F32, tag="rowf", name=f"sd{tag}{kt}")
            nc.scalar.activation(out=sd, in_=var, func=AF.Sqrt, bias=eps_t,
                                 scale=1.0)
            r = rpool.tile([1, 512], F32, tag="rowf", name=f"r{tag}{kt}")
            nc.vector.reciprocal(out=r, in_=sd)
            rbfr = rpool.tile([1, 512], BF16, tag="rowb", name=f"rbfr{tag}{kt}")
            nc.vector.tensor_copy(out=rbfr, in_=r)
            mur = rpool.tile([1, 512], F32, tag="rowf", name=f"mur{tag}{kt}")
            nc.vector.tensor_mul(mur, mu, r)
            rbfm = rpool.tile([1, 512], BF16, tag="rowb", name=f"rbfm{tag}{kt}")
            nc.vector.tensor_copy(out=rbfm, in_=mur)
            bp1 = pp_b.tile([P, 512], F32, tag="bc", name=f"bp1{tag}{kt}")
            nc.tensor.matmul(bp1, onesK1, rbfr)
            nc.scalar.copy(out=rB[:, ts(kt, 512)], in_=bp1)
            bp2 = pp_b.tile([P, 512], F32, tag="bc", name=f"bp2{tag}{kt}")
            nc.tensor.matmul(bp2, onesK1, rbfm)
            nc.scalar.copy(out=murB[:, ts(kt, 512)], in_=bp2)

    # attention-output chunks, produced in D, consumed by proj in E
    poch = tc.alloc_tile_pool(name="poch", bufs=8)
    # pbig: time-shared 4MB-class slots (tag s4):
    #   A: xT, xsq, xln | D: xln, P(even), P(odd)
    pbig = tc.alloc_tile_pool(name="pbig", bufs=3)

    # ============ Phase A: x^T load, LN1 -> xln ============
    prbm = tc.alloc_tile_pool(name="prbm", bufs=1)
    rowsA = tc.alloc_tile_pool(name="rowsA", bufs=8)
    ppA_st = tc.alloc_tile_pool(name="ppA_st", bufs=2, space="PSUM")
    ppA_b = tc.alloc_tile_pool(name="ppA_b", bufs=2, space="PSUM")

    xT = pbig.tile([P, DC, T], BF16, tag="s4", name="xT")
    for c in range(DC):
        for hh in range(2):
            nc.sync.dma_start(out=xT[:, c, ts(hh, T // 2)],
                              in_=xT_d[c][:, ts(hh, T // 2)])
    xsq = pbig.tile([P, DC, T], BF16, tag="s4", name="xsq")
    for c in range(DC):
        nc.vector.tensor_mul(xsq[:, c, :], xT[:, c, :], xT[:, c, :])
    rB = prbm.tile([P, T], BF16, tag="rb", name="rB")
    murB = prbm.tile([P, T], BF16, tag="mb", name="murB")
    ln_stats_bcast(ppA_st, ppA_b, rowsA, xT, xsq, T, rB, murB, "1")
    xln = pbig.tile([P, DC, T], BF16, tag="s4", name="xln")
    for c in range(DC):
        nc.vector.tensor_mul(xln[:, c, :], xT[:, c, :], rB)
        nc.vector.tensor_sub(xln[:, c, :], xln[:, c, :], murB)

    ppA_b.release()
    ppA_st.release()
    rowsA.release()
    prbm.release()

    # ===== Phases C+D fused: V, then per head-pair K,Q -> scores -> exp
    # -> PV -> normalize, software-pipelined so PE matmuls overlap ACT exp.
    pvaug = tc.alloc_tile_pool(name="pvaug", bufs=1)
    pwkv = tc.alloc_tile_pool(name="pwkv", bufs=3)
    pKp = tc.alloc_tile_pool(name="pKp", bufs=2)
    pQp = tc.alloc_tile_pool(name="pQp", bufs=2)
    poun = tc.alloc_tile_pool(name="poun", bufs=2)
    prb = tc.alloc_tile_pool(name="prb", bufs=2)
    ppD_mm = tc.alloc_tile_pool(name="ppD_mm", bufs=2, space="PSUM")
    ppD_s = tc.alloc_tile_pool(name="ppD_s", bufs=2, space="PSUM")
    ppD_ob = tc.alloc_tile_pool(name="ppD_ob", bufs=2, space="PSUM")
    pwv = tc.alloc_tile_pool(name="pwv", bufs=1)

    def make_kq(hp):
        """Produce the pair's K^T [P, T] and Q^T [P, Tq] chunks."""
        wkj = pwkv.tile([P, DC, P], BF16, tag="w", name=f"wkj{hp}")
        nc.sync.dma_start(out=wkj, in_=io["wk"][hp])
        Kp = pKp.tile([P, T], BF16, tag="kp", name=f"kp{hp}")
        for npair in range(NT // 2):
            ps2 = [ppD_mm.tile([P, 512], F32, tag="mm",
                               name=f"psk{hp}_{npair}_{n}") for n in range(2)]
            for k in range(DC):
                for n in range(2):
                    nc.tensor.matmul(
                        ps2[n], wkj[:, k, :],
                        xln[:, k, ts(2 * npair + n, 512)],
                        start=(k == 0), stop=(k == DC - 1))
            for n in range(2):
                nc.vector.tensor_scalar_add(
                    Kp[:, ts(2 * npair + n, 512)], ps2[n],
                    bqkv_s[:, DC + hp:DC + hp + 1])
        wqj = pwkv.tile([P, DC, P], BF16, tag="w", name=f"wqj{hp}")
        nc.sync.dma_start(out=wqj, in_=io["wq"][hp])
        Qp = pQp.tile([P, Tq], BF16, tag="qp", name=f"qp{hp}")
        ps2 = [ppD_mm.tile([P, 512], F32, tag="mm", name=f"psq{hp}_{n}")
               for n in range(NQ)]
        for k in range(DC):
            for n in range(NQ):
                nc.tensor.matmul(ps2[n], wqj[:, k, :], xln[:, k, ts(n, 512)],
                                 start=(k == 0), stop=(k == DC - 1))
        for n in range(NQ):
            nc.vector.tensor_scalar_add(Qp[:, ts(n, 512)], ps2[n],
                                        bqkv_s[:, hp:hp + 1])
        return Kp, Qp

    och_tiles = []
    kq = make_kq(0)

    wv_t = pwv.tile([P, DC, D], BF16, name="wv_t")
    for c in range(DC):
        nc.sync.dma_start(out=wv_t[:, c, :], in_=io["wv"][c])
    v_aug = pvaug.tile([P, TKC, H * (HS + 1)], BF16, name="v_aug")
    v4 = v_aug.rearrange("p i (h e) -> p i h e", e=HS + 1)
    nc.vector.memset(v4[:, :, :, HS:HS + 1], 1.0)
    for i in range(TKC):
        ps = [ppD_mm.tile([P, 512], F32, tag="mm", name=f"psv{i}_{n}")
              for n in range(NQ)]
        for k in range(DC):
            for n in range(NQ):
                nc.tensor.matmul(ps[n], xln[:, k, ts(i, P)],
                                 wv_t[:, k, ts(n, 512)],
                                 start=(k == 0), stop=(k == DC - 1))
        for n in range(NQ):
            dst = v4[:, i, n * DC:(n + 1) * DC, 0:HS]
            nc.vector.tensor_add(dst,
                                 ps[n].rearrange("p (h d) -> p h d", d=HS),
                                 bvB[:, ts(n, 512)].rearrange(
                                     "p (h d) -> p h d", d=HS))
    pwv.release()

    for hp in range(DC):
        Kp, Qp = kq
        p_tiles = {}
        for local in (0, 1):
            p_tiles[local] = pbig.tile([P, TKC, Tq], BF16, tag="s4",
                                       name=f"pt{hp}_{local}")
        # scores + exp, two heads interleaved (row-tiled on PE)
        for kc in range(TKC):
            ps_s = {}
            for local in (0, 1):
                lo = local * HS
                ps_s[local] = ppD_s.tile([P, Tq], F32, tag="s",
                                         name=f"pss{hp}_{kc}_{local}")
                for n in range(NQ):
                    nc.tensor.matmul(ps_s[local][:, ts(n, 512)],
                                     Kp[lo:lo + HS, ts(kc, P)],
                                     Qp[lo:lo + HS, ts(n, 512)])
            for local in (0, 1):
                nc.scalar.activation(out=p_tiles[local][:, kc, :],
                                     in_=ps_s[local], func=AF.Exp,
                                     scale=float(1.0 / np.sqrt(HS)))
        # produce next pair's K/Q while ACT exps this pair
        if hp + 1 < DC:
            kq = make_kq(hp + 1)
        # PV + denominators
        oun = poun.tile([P, Tq], BF16, tag="ou", name=f"oun{hp}")
        recips_bf = {}
        for local in (0, 1):
            h = 2 * hp + local
            rc = prb.tile([1, Tq], F32, tag="rc", name=f"rc{hp}_{local}")
            for qt in range(NQ):
                po = ppD_ob.tile([HS + 1, 512], F32, tag="ob",
                                 name=f"po{h}_{qt}")
                for kc in range(TKC):
                    nc.tensor.matmul(
                        po, v_aug[:, kc, h * (HS + 1):(h + 1) * (HS + 1)],
                        p_tiles[local][:, kc, ts(qt, 512)],
                        start=(kc == 0), stop=(kc == TKC - 1))
                nc.vector.tensor_copy(
                    out=oun[local * HS:(local + 1) * HS, ts(qt, 512)],
                    in_=po[0:HS, :])
                nc.vector.reciprocal(out=rc[:, ts(qt, 512)],
                                     in_=po[HS:HS + 1, :])
            rcb = prb.tile([1, Tq], BF16, tag="rcb", name=f"rcb{hp}_{local}")
            nc.vector.tensor_copy(out=rcb, in_=rc)
            recips_bf[local] = rcb
        # broadcast each head's reciprocal row to its 64-partition half
        och = poch.tile([P, Tq], BF16, tag="oc", name=f"och{hp}")
        for n in range(NQ):
            rbp = ppD_ob.tile([P, 512], F32, tag="ob", name=f"rbp{hp}_{n}")
            for local in (0, 1):
                nc.tensor.matmul(rbp[local * HS:(local + 1) * HS, :],
                                 onesK1[:, 0:HS],
                                 recips_bf[local][:, ts(n, 512)])
            rbs = prb.tile([P, 512], BF16, tag="rbs", name=f"rbs{hp}_{n}")
            nc.vector.tensor_copy(out=rbs, in_=rbp)
            nc.vector.tensor_mul(och[:, ts(n, 512)], oun[:, ts(n, 512)], rbs)
        och_tiles.append(och)

    ppD_ob.release()
    ppD_s.release()
    ppD_mm.release()
    prb.release()
    poun.release()
    pQp.release()
    pKp.release()
    pwkv.release()
    pvaug.release()
    pbig.release()

    # ============ Phase E: proj + residual -> h (SBUF), n-outer so LN2
    # stats can start after the first column half is complete ============
    ph = tc.alloc_tile_pool(name="ph", bufs=1)
    pxq = tc.alloc_tile_pool(name="pxq", bufs=1)
    pwproj = tc.alloc_tile_pool(name="pwproj", bufs=8)
    ppE = tc.alloc_tile_pool(name="ppE", bufs=8, space="PSUM")

    h_t = ph.tile([P, DC, Tq], F32, name="h_t")
    wpj = []
    for j in range(DC):
        wj = pwproj.tile([P, DC, P], BF16, tag="w", name=f"wpj{j}")
        nc.sync.dma_start(out=wj, in_=io["wproj"][j])
        wpj.append(wj)
    xq_t = pxq.tile([P, DC, Tq], BF16, name="xq_t")
    for c in range(DC):
        nc.sync.dma_start(out=xq_t[:, c, :], in_=xT_d[c][:, 0:Tq])
    for n in range(NQ):
        for j in range(DC):
            psn = ppE.tile([P, 512], F32, tag="mm", name=f"psp{j}_{n}")
            for k in range(DC):
                nc.tensor.matmul(psn, wpj[j][:, k, :],
                                 och_tiles[k][:, ts(n, 512)],
                                 start=(k == 0), stop=(k == DC - 1))
            nc.vector.scalar_tensor_tensor(
                out=h_t[:, j, ts(n, 512)], in0=psn,
                scalar=bproj_s[:, j:j + 1], in1=xq_t[:, j, ts(n, 512)],
                op0=ALU.add, op1=ALU.add)
    ppE.release()
    pwproj.release()
    pxq.release()

    # ============ Phase F: LN2 + gelu -> g ============
    pg = tc.alloc_tile_pool(name="pg", bufs=1)
    phb = tc.alloc_tile_pool(name="phb", bufs=1)
    phsq = tc.alloc_tile_pool(name="phsq", bufs=1)
    pcen = tc.alloc_tile_pool(name="pcen", bufs=1)
    rowsF = tc.alloc_tile_pool(name="rowsF", bufs=8)
    pr2 = tc.alloc_tile_pool(name="pr2", bufs=1)
    ppF_st = tc.alloc_tile_pool(name="ppF_st", bufs=4, space="PSUM")
    ppF_b = tc.alloc_tile_pool(name="ppF_b", bufs=4, space="PSUM")

    g_t = pg.tile([P, DC, Tq], BF16, name="g_t")
    hb = phb.tile([P, DC, Tq], BF16, name="hb")
    hsq = phsq.tile([P, DC, Tq], BF16, name="hsq")
    for kt in range(NQ):
        for c in range(DC):
            nc.vector.tensor_copy(out=hb[:, c, ts(kt, 512)],
                                  in_=h_t[:, c, ts(kt, 512)])
            nc.vector.tensor_mul(hsq[:, c, ts(kt, 512)],
                                 hb[:, c, ts(kt, 512)],
                                 hb[:, c, ts(kt, 512)])
    r2B = pr2.tile([P, Tq], BF16, tag="rb", name="r2B")
    mur2B = pr2.tile([P, Tq], BF16, tag="mb", name="mur2B")
    ln_stats_bcast(ppF_st, ppF_b, rowsF, hb, hsq, Tq, r2B, mur2B, "2")
    cen = pcen.tile([P, DC, Tq], BF16, name="cen")
    for kt in range(NQ):
        for c in range(DC):
            nc.vector.tensor_mul(cen[:, c, ts(kt, 512)],
                                 hb[:, c, ts(kt, 512)], r2B[:, ts(kt, 512)])
            nc.vector.tensor_sub(cen[:, c, ts(kt, 512)],
                                 cen[:, c, ts(kt, 512)],
                                 mur2B[:, ts(kt, 512)])
            nc.scalar.activation(out=g_t[:, c, ts(kt, 512)],
                                 in_=cen[:, c, ts(kt, 512)], func=AF.Gelu,
                                 bias=bln2_s[:, c:c + 1],
                                 scale=g2_s[:, c:c + 1])
    ppF_b.release()
    ppF_st.release()
    pr2.release()
    rowsF.release()
    pcen.release()
    phsq.release()
    phb.release()

    # ============ Phase G: FFN ============
    pf1 = tc.alloc_tile_pool(name="pf1", bufs=1)
    ppG = tc.alloc_tile_pool(name="ppG", bufs=8, space="PSUM")
    pw2 = tc.alloc_tile_pool(name="pw2", bufs=2)
    pw1 = tc.alloc_tile_pool(name="pw1", bufs=3)

    w2_first = pw2.tile([P, FC, P], BF16, tag="w2", name="w2t0")
    nc.sync.dma_start(out=w2_first, in_=io["w2"][0])
    f1g = pf1.tile([P, FC, Tq], BF16, name="f1g")
    for j in range(FC):
        w1_t = pw1.tile([P, DC, P], BF16, tag="w1", name=f"w1t{j}")
        nc.sync.dma_start(out=w1_t, in_=io["w1"][j])
        ps = [ppG.tile([P, 512], F32, tag="mm", name=f"psf{j}_{n}")
              for n in range(NQ)]
        for k in range(DC):
            for n in range(NQ):
                nc.tensor.matmul(ps[n], w1_t[:, k, :], g_t[:, k, ts(n, 512)],
                                 start=(k == 0), stop=(k == DC - 1))
        for n in range(NQ):
            nc.scalar.activation(out=f1g[:, j, ts(n, 512)], in_=ps[n],
                                 func=AF.Gelu, bias=b1_s[:, j:j + 1],
                                 scale=1.0)
    pw1.release()

    poutc = tc.alloc_tile_pool(name="poutc", bufs=2)
    for j in range(DC):
        if j == 0:
            w2_t = w2_first
        else:
            w2_t = pw2.tile([P, FC, P], BF16, tag="w2", name=f"w2t{j}")
            nc.sync.dma_start(out=w2_t, in_=io["w2"][j])
        ps = [ppG.tile([P, 512], F32, tag="mm", name=f"pso{j}_{n}")
              for n in range(NQ)]
        for k in range(FC):
            for n in range(NQ):
                nc.tensor.matmul(ps[n], w2_t[:, k, :], f1g[:, k, ts(n, 512)],
                                 start=(k == 0), stop=(k == FC - 1))
        outc = poutc.tile([P, Tq], F32, tag="oc", name=f"outc{j}")
        for n in range(NQ):
            nc.vector.scalar_tensor_tensor(
                out=outc[:, ts(n, 512)], in0=ps[n], scalar=b2_s[:, j:j + 1],
                in1=h_t[:, j, ts(n, 512)], op0=ALU.add, op1=ALU.add)
            nc.sync.dma_start(out=io["out"][j][:, ts(n, 512)],
                              in_=outc[:, ts(n, 512)])

    poutc.release()
    pw2.release()
    ppG.release()
    pf1.release()
    pg.release()
    ph.release()
    poch.release()
    consts.release()


# ----------------------------------------------------------------------------
# host side
# ----------------------------------------------------------------------------

def _stripe(v):
    """[n*P] -> [P, n] per-partition striping (feature f = c*P + p)."""
    v = np.asarray(v, np.float32)
    return np.ascontiguousarray(v.reshape(-1, P).T)


def _lhsT_stream(W):
    """[Din, Dout] -> [Dout/P, P, Din/P, P] so slice [j] is the lhsT stream
    tile [P(din), Din/P, P(dout cols)] with contiguous per-partition rows."""
    din, dout = W.shape
    r = W.astype(bf16).reshape(din // P, P, dout // P, P)
    return np.ascontiguousarray(r.transpose(2, 1, 0, 3))


def prep_shared(inputs):
    f32 = np.float32
    g1 = np.asarray(inputs["ln1_g"], f32)
    b1n = np.asarray(inputs["ln1_b"], f32)
    W_ap = np.asarray(inputs["W_ap"], f32)
    b_ap = np.asarray(inputs["b_ap"], f32)
    W_qkv = np.asarray(inputs["W_qkv"], f32)
    b_qkv = np.asarray(inputs["b_qkv"], f32)
    W_proj = np.asarray(inputs["W_proj"], f32)

    # fold LN1 gamma and the whole attn pre-projection into W_qkv:
    # qkv = ln1(x) @ W_ap' @ W_qkv + (b_ap' @ W_qkv + b_qkv)
    W_eff = (g1[:, None] * W_ap) @ W_qkv
    b_eff = (b_ap + b1n @ W_ap) @ W_qkv + b_qkv
    shared = {
        "wq": _lhsT_stream(W_eff[:, 0:D]),
        "wk": _lhsT_stream(W_eff[:, D:2 * D]),
        "wv": np.ascontiguousarray(
            W_eff[:, 2 * D:].astype(bf16).reshape(DC, P, D)),
        "bqkv": _stripe(b_eff[:2 * D]),
        "bv": np.ascontiguousarray(np.asarray(b_eff[2 * D:], f32)),
        "wproj": _lhsT_stream(W_proj),
        "bproj": _stripe(np.asarray(inputs["b_proj"], f32)),
        "w1": _lhsT_stream(np.asarray(inputs["W1"], f32)),
        "b1": _stripe(np.asarray(inputs["b1"], f32)),
        "w2": _lhsT_stream(np.asarray(inputs["W2"], f32)),
        "b2": _stripe(np.asarray(inputs["b2"], f32)),
        "g2": _stripe(np.asarray(inputs["ln2_g"], f32)),
        "bln2": _stripe(np.asarray(inputs["ln2_b"], f32)),
    }
    return shared


def prep_core_x(x, core):
    b, qh = core // 2, core % 2
    xTb = np.asarray(x[b], np.float32).T  # [D, T] view
    if qh:
        xTb = np.concatenate([xTb[:, Tq:], xTb[:, :Tq]], axis=1)
    return np.ascontiguousarray(xTb.astype(bf16).reshape(DC, P, T))


def assemble_output(results, dtype):
    out = np.empty((B, T, D), dtype)
    for c in range(N_CORES):
        b, qh = c // 2, c % 2
        arr = np.asarray(results[c]["out"]).reshape(D, Tq)
        out[b, qh * Tq:(qh + 1) * Tq, :] = arr.T
    return out


def kernel(**inputs):
    x = np.asarray(inputs["x"], np.float32)
    shared = prep_shared(inputs)
    nc = build_nc()
    in_maps = [dict(shared, xt=prep_core_x(x, c)) for c in range(N_CORES)]
    res = run_bass_kernel_spmd(nc, in_maps, list(range(N_CORES)))
    return assemble_output(res.results, np.float32)


if __name__ == "__main__":
    nc = build_nc()
    print("built ok")



# revision 20
# speedup vs baseline: 1.0160x; 1.0160x over previous
"""Trainium2 Bass kernel for a dense transformer block (nn_Block_7713761264306).

Sharding: 8 cores = 4 batches x 2 query-halves (token-rotation, exact).
All heavy GEMMs run as fp8e4m3 DoubleRow matmuls (two k-tiles per
instruction at 0.5 cycles/row -> 4x bf16 throughput):
  - QKV/proj/FFN: contraction pairs = adjacent feature chunks.
  - scores: per-head HS=64 contraction split as two 32-row k-tiles; K/Q are
    packed 4 heads per 128 partitions ([32*slot, half, token]) via a
    host-side within-chunk column permutation of W_qkv.
  - PV: contraction pairs = adjacent 128-key chunks of the probs tiles;
    denominators ride along as a ones-column in v_aug (M=65).
Precision recovery (rel-err budget ~1.2e-2 < 2e-2 gate):
  - FFN1: W1 hi + W1 lo(x64) chains, combined (A + B/64) on GpSimd.
  - FFN2: f1 kept as bf16->fp8(hi)+fp8(residual) exact split (both chains
    accumulate into the same PSUM), plus W2 lo(x64) chain.
LayerNorm statistics stay bf16 on the PE (ones-matmuls); probs are
exp(s/8 - 3) in fp8e4 (the global shift cancels in normalization).
"""

import numpy as np
import ml_dtypes

import concourse.bass as bass
import concourse.mybir as mybir
import concourse.tile as tile
from concourse.bass import ts
from concourse.bass_utils import run_bass_kernel_spmd

BF16 = mybir.dt.bfloat16
F32 = mybir.dt.float32
FP8 = mybir.dt.float8e4
bf16 = ml_dtypes.bfloat16
e4m3 = ml_dtypes.float8_e4m3

B, T, D, H, HS, FF = 4, 2048, 1024, 16, 64, 4096
P = 128
DC = D // P          # 8 feature chunks
DP = DC // 2         # 4 feature-chunk pairs
FC = FF // P         # 32 ffn chunks
FP = FC // 2         # 16 ffn chunk pairs
TKC = T // P         # 16 key chunks
KP = TKC // 2        # 8 key-chunk pairs
Tq = 1024            # queries per core
NT = T // 512        # 4 column tiles over full seq
NQ = Tq // 512       # 2 column tiles over queries
NG = 4               # head groups (4 heads each)
N_CORES = 8
EPS = 1e-5
CSH = 3.0            # exp shift: probs = exp(s/8 - CSH), cancels in softmax
LO_SCALE = 64.0

AF = mybir.ActivationFunctionType
ALU = mybir.AluOpType
DR = mybir.MatmulPerfMode.DoubleRow
DEBUG = False

# precision chains (error budget knobs)
USE_W1LO = True      # kills ffn1 W-side quantization error
USE_GSPLIT = True    # kills ffn1 x-side error (g = gh + gr exact split)
USE_F1SPLIT = True   # kills ffn2 x-side error
USE_W2LO = True      # kills ffn2 W-side error
# exp offload: which (h, kp) pair-tiles go to Pool (Schraudolph + convert)
POOL_EXP_MOD = 10 ** 9  # Pool exp offload disabled (serialization cost)
SCHR_K = 128.0 / np.log(2.0)   # bf16 Schraudolph slope
SCHR_B = 16249.0               # calibrated offset (rms 1.8%)


def build_nc(cap=True):
    nc = bass.Bass()
    io = {}
    io["xt"] = nc.dram_tensor("xt", [DC, P, T], BF16, kind="ExternalInput")
    io["wq"] = nc.dram_tensor("wq", [DC, P, 4, 2, P], FP8, kind="ExternalInput")
    io["wk"] = nc.dram_tensor("wk", [DC, P, 4, 2, P], FP8, kind="ExternalInput")
    io["wv"] = nc.dram_tensor("wv", [P, DC, D], FP8, kind="ExternalInput")
    io["bqk"] = nc.dram_tensor("bqk", [P, 2 * DC], F32, kind="ExternalInput")
    io["bv"] = nc.dram_tensor("bv", [D], F32, kind="ExternalInput")
    io["wproj"] = nc.dram_tensor("wproj", [DC, P, 4, 2, P], FP8,
                                 kind="ExternalInput")
    io["bproj"] = nc.dram_tensor("bproj", [P, DC], F32, kind="ExternalInput")
    io["w1h"] = nc.dram_tensor("w1h", [FC, P, 4, 2, P], FP8,
                               kind="ExternalInput")
    io["w1l"] = nc.dram_tensor("w1l", [FC, P, 4, 2, P], FP8,
                               kind="ExternalInput")
    io["b1"] = nc.dram_tensor("b1", [P, FC], F32, kind="ExternalInput")
    io["w2h"] = nc.dram_tensor("w2h", [DC, P, 16, 2, P], FP8,
                               kind="ExternalInput")
    io["w2l"] = nc.dram_tensor("w2l", [DC, P, 16, 2, P], FP8,
                               kind="ExternalInput")
    io["b2"] = nc.dram_tensor("b2", [P, DC], F32, kind="ExternalInput")
    io["g2"] = nc.dram_tensor("g2", [P, DC], F32, kind="ExternalInput")
    io["bln2"] = nc.dram_tensor("bln2", [P, DC], F32, kind="ExternalInput")
    io["zeros"] = nc.dram_tensor("zeros", [T], FP8, kind="ExternalInput")
    io["out"] = nc.dram_tensor("out", [DC, P, Tq], F32, kind="ExternalOutput")
    if DEBUG:
        for nm, shp, dt in [
                ("d_xln", [P, DC, T], FP8), ("d_k0", [P, 2, T], FP8),
                ("d_q0", [P, 2, Tq], FP8), ("d_vaug", [P, TKC, H * (HS + 1)], FP8),
                ("d_p0", [P, TKC, Tq], FP8), ("d_och", [P, DC, Tq], FP8),
                ("d_ht", [P, DC, Tq], F32), ("d_g8", [P, DC, Tq], FP8),
                ("d_f1h", [P, FC, Tq], FP8), ("d_f1r", [P, FC, Tq], FP8)]:
            io[nm] = nc.dram_tensor(nm, shp, dt, kind="ExternalOutput")

    with tile.TileContext(nc) as tc:
        _emit(nc, tc, io)
    nc.finalize()
    if cap:
        _cap_waits(nc)
    return nc


def _cap_waits(nc, keep_types=()):
    """This toolchain's walrus accepts only one sync-wait command per compute
    instruction; hoist extra waits into preceding same-engine NoOps."""
    cnt = 0
    for fn in nc.m.functions:
        for blk in fn.blocks:
            new = []
            for inst in blk.instructions:
                si = getattr(inst, "sync_info", None)
                if si is not None and len(si.on_wait) > 1 \
                        and type(inst).__name__ not in keep_types:
                    waits = list(si.on_wait)
                    for w in waits[:-1]:
                        cnt += 1
                        nop = mybir.InstNoOp(
                            name=f"{inst.name}-w{cnt}", ins=[], outs=[])
                        nop.engine = inst.engine
                        nop.sync_info = mybir.SyncInfo(on_wait=[w],
                                                       on_update=[])
                        new.append(nop)
                    inst.sync_info = mybir.SyncInfo(
                        on_wait=[waits[-1]], on_update=list(si.on_update))
                new.append(inst)
            blk.instructions = new
    return cnt


def _emit(nc, tc, io):
    xT_d = io["xt"]
    consts = tc.alloc_tile_pool(name="consts", bufs=1)

    # ------------------------- constants -------------------------
    bqk_s = consts.tile([P, 2 * DC], F32)
    nc.sync.dma_start(out=bqk_s, in_=io["bqk"][:])
    bproj_s = consts.tile([P, DC], F32)
    nc.sync.dma_start(out=bproj_s, in_=io["bproj"][:])
    b1_s = consts.tile([P, FC], F32)
    nc.sync.dma_start(out=b1_s, in_=io["b1"][:])
    b2_s = consts.tile([P, DC], F32)
    nc.sync.dma_start(out=b2_s, in_=io["b2"][:])
    g2_s = consts.tile([P, DC], F32)
    nc.sync.dma_start(out=g2_s, in_=io["g2"][:])
    bln2_s = consts.tile([P, DC], F32)
    nc.sync.dma_start(out=bln2_s, in_=io["bln2"][:])
    bvB = consts.tile([P, D], F32)
    nc.sync.dma_start(out=bvB, in_=io["bv"][:].partition_broadcast(P))

    invD = consts.tile([P, 1], BF16)
    nc.vector.memset(invD, 1.0 / D)
    onesK1 = consts.tile([1, P], BF16)
    nc.vector.memset(onesK1, 1.0)
    eps_t = consts.tile([1, 1], F32)
    nc.vector.memset(eps_t, EPS)
    negc_t = consts.tile([P, 1], F32)
    nc.vector.memset(negc_t, -CSH)

    def ln_stats_bcast(pp_stat, pp_b, rpool, src_bf, sq_fn, ncols, rB, murB,
                       tag):
        """Per-512 col block: mean/E[x^2] over partitions via PE ones-matmul,
        row math, then broadcast 1/sd and mu/sd to [P, 512] via K=1 matmul.
        sq_fn(c, kt) returns the [P, 512] x^2 AP for chunk c, block kt."""
        for kt in range(ncols // 512):
            ps_mu = pp_stat.tile([1, 512], F32, tag="st",
                                 name=f"psmu{tag}{kt}")
            for c in range(DC):
                nc.tensor.matmul(ps_mu, invD, src_bf[:, c, ts(kt, 512)],
                                 start=(c == 0), stop=(c == DC - 1))
            ps_sq = pp_stat.tile([1, 512], F32, tag="st",
                                 name=f"pssq{tag}{kt}")
            for c in range(DC):
                nc.tensor.matmul(ps_sq, invD, sq_fn(c, kt),
                                 start=(c == 0), stop=(c == DC - 1))
            mu = rpool.tile([1, 512], F32, tag="rowf", name=f"mu{tag}{kt}")
            nc.vector.tensor_copy(out=mu, in_=ps_mu)
            var = rpool.tile([1, 512], F32, tag="rowf", name=f"var{tag}{kt}")
            nc.vector.tensor_mul(var, mu, mu)
            nc.vector.tensor_sub(var, ps_sq, var)
            sd = rpool.tile([1, 512], F32, tag="rowf", name=f"sd{tag}{kt}")
            nc.scalar.activation(out=sd, in_=var, func=AF.Sqrt, bias=eps_t,
                                 scale=1.0)
            r = rpool.tile([1, 512], F32, tag="rowf", name=f"r{tag}{kt}")
            nc.vector.reciprocal(out=r, in_=sd)
            rbfr = rpool.tile([1, 512], BF16, tag="rowb", name=f"rbfr{tag}{kt}")
            nc.vector.tensor_copy(out=rbfr, in_=r)
            mur = rpool.tile([1, 512], F32, tag="rowf", name=f"mur{tag}{kt}")
            nc.vector.tensor_mul(mur, mu, r)
            rbfm = rpool.tile([1, 512], BF16, tag="rowb", name=f"rbfm{tag}{kt}")
            nc.vector.tensor_copy(out=rbfm, in_=mur)
            bp1 = pp_b.tile([P, 512], F32, tag="bc", name=f"bp1{tag}{kt}")
            nc.tensor.matmul(bp1, onesK1, rbfr)
            nc.scalar.copy(out=rB[:, ts(kt, 512)], in_=bp1)
            bp2 = pp_b.tile([P, 512], F32, tag="bc", name=f"bp2{tag}{kt}")
            nc.tensor.matmul(bp2, onesK1, rbfm)
            nc.scalar.copy(out=murB[:, ts(kt, 512)], in_=bp2)

    # Persistent pools (allocated below phase-A scratch so the LIFO stack
    # lets xln8 die right after the qb0 attention group).
    ph = tc.alloc_tile_pool(name="ph", bufs=1)
    h_t = ph.tile([P, DC, Tq], BF16, name="h_t")
    poch = tc.alloc_tile_pool(name="poch", bufs=1)
    och8 = poch.tile([P, DC, Tq], FP8, name="och8")
    pg = tc.alloc_tile_pool(name="pg", bufs=1)
    g8 = pg.tile([P, DC, 512], FP8, name="g8")
    gr8 = pg.tile([P, DC, 512], FP8, name="gr8")
    g64 = pg.tile([P, DC, 512], FP8, name="g64")
    pkq = tc.alloc_tile_pool(name="pkq", bufs=1)
    pvaug = tc.alloc_tile_pool(name="pvaug", bufs=1)

    # K: one plain tile; DoubleRow tile-1 reads the NEXT chunk's (real,
    # finite) data, which the zeroed Q tile-1 multiplies away. Chunk DC is a
    # zeroed pad so c = DC-1 stays in bounds.
    Kall = pkq.tile([P, DC + 1, T], FP8, name="Kall")
    nc.sync.dma_start(out=Kall[:, DC, :],
                      in_=io["zeros"][:].partition_broadcast(P))
    Q4 = [pkq.tile([P, 2, Tq], FP8, name=f"Q4_{c}") for c in range(DC)]
    for c in range(DC):
        nc.sync.dma_start(out=Q4[c][:, 1, :],
                          in_=io["zeros"][0:Tq].partition_broadcast(P))
    v_aug = pvaug.tile([P, TKC, H * (HS + 1)], FP8, name="v_aug")
    v4 = v_aug.rearrange("p i (h e) -> p i h e", e=HS + 1)
    nc.vector.memset(v4[:, :, :, HS:HS + 1], 1.0)

    # ============ Phase A: x^T load, LN1 -> xln8 (fp8) ============
    pxln = tc.alloc_tile_pool(name="pxln", bufs=1)
    pbigA = tc.alloc_tile_pool(name="pbigA", bufs=2)
    prbm = tc.alloc_tile_pool(name="prbm", bufs=2)
    rowsA = tc.alloc_tile_pool(name="rowsA", bufs=4)
    ppA_st = tc.alloc_tile_pool(name="ppA_st", bufs=2, space="PSUM")
    ppA_b = tc.alloc_tile_pool(name="ppA_b", bufs=2, space="PSUM")
    ptmp = tc.alloc_tile_pool(name="ptmp", bufs=2)

    xln8 = pxln.tile([P, DC, T], FP8, name="xln8")
    # block-local LN1: per 512-token block, stream x in, compute stats,
    # normalize, emit fp8
    for kt in range(NT):
        xTb = pbigA.tile([P, DC, 512], BF16, tag="s4", name=f"xT{kt}")
        for c in range(DC):
            nc.sync.dma_start(out=xTb[:, c, :], in_=xT_d[c][:, ts(kt, 512)])

        def xsq_fn(c, _kt, _x=xTb):
            sq = ptmp.tile([P, 512], BF16, tag="sq", name=f"xsq{c}_{kt}")
            nc.vector.tensor_mul(sq, _x[:, c, :], _x[:, c, :])
            return sq

        rB = prbm.tile([P, 512], BF16, tag="rb", name=f"rB{kt}")
        murB = prbm.tile([P, 512], BF16, tag="mb", name=f"murB{kt}")
        ln_stats_bcast(ppA_st, ppA_b, rowsA, xTb, xsq_fn, 512, rB, murB,
                       f"1{kt}")
        for c in range(DC):
            tmp = ptmp.tile([P, 512], BF16, tag="t", name=f"xlt{kt}_{c}")
            nc.vector.tensor_mul(tmp, xTb[:, c, :], rB)
            nc.vector.tensor_sub(xln8[:, c, ts(kt, 512)], tmp, murB)

    ptmp.release()
    ppA_b.release()
    ppA_st.release()
    rowsA.release()
    prbm.release()
    pbigA.release()

    # ===== attention + proj/LN2/FFN, query-block pipelined =====
    schr_s = float(0.125 * SCHR_K)
    schr_b = float(SCHR_B - CSH * SCHR_K)

    for qb in range(NQ):
        # ---------------- attention group for this query block ----------
        if qb == 0:
            pwkv = tc.alloc_tile_pool(name="pwkv", bufs=4)
            pp_kqv = tc.alloc_tile_pool(name="pp_kqv", bufs=1, space="PSUM")
            pwv = tc.alloc_tile_pool(name="pwv", bufs=1)
        else:
            pwf = tc.alloc_tile_pool(name="pwf", bufs=2)
            pfb = tc.alloc_tile_pool(name="pfb", bufs=3)
            pfoc = tc.alloc_tile_pool(name="pfoc", bufs=2)
            ppFF = tc.alloc_tile_pool(name="ppFF", bufs=1, space="PSUM")
        pprob = tc.alloc_tile_pool(name="pprob", bufs=2)
        pi16p = tc.alloc_tile_pool(name="pi16p", bufs=1)
        prb = tc.alloc_tile_pool(name="prb", bufs=2)
        ppS = tc.alloc_tile_pool(name="ppS", bufs=2, space="PSUM")
        ppOB = tc.alloc_tile_pool(name="ppOB", bufs=1, space="PSUM")
        ppRB = tc.alloc_tile_pool(name="ppRB", bufs=1, space="PSUM")

        def emit_kq_chunk(c, which):
            w_d = io["wk"] if which == "k" else io["wq"]
            nblk = NT if which == "k" else NQ
            bcol = DC + c if which == "k" else c
            wt = pwkv.tile([P, 4, 2, P], FP8, tag="w", name=f"w{which}{c}")
            nc.sync.dma_start(out=wt, in_=w_d[c])
            for nt in range(nblk):
                ps = pp_kqv.tile([P, 512], F32, tag="mm",
                                 name=f"ps{which}{c}_{nt}")
                for t in range(4):
                    nc.tensor.matmul(ps, wt[:, t],
                                     xln8[:, 2 * t:2 * t + 2, ts(nt, 512)],
                                     start=(t == 0), stop=(t == 3),
                                     perf_mode=DR)
                dst = (Kall[:, c, ts(nt, 512)] if which == "k"
                       else Q4[c][:, 0, ts(nt, 512)])
                nc.vector.tensor_scalar_add(dst, ps,
                                            bqk_s[:, bcol:bcol + 1])

        def emit_v_chunk(i):
            for n in range(NQ):
                ps = pp_kqv.tile([P, 512], F32, tag="mm", name=f"psv{i}_{n}")
                for t in range(4):
                    nc.tensor.matmul(ps, xln8[:, 2 * t:2 * t + 2, ts(i, P)],
                                     wv_t[:, 2 * t:2 * t + 2, ts(n, 512)],
                                     start=(t == 0), stop=(t == 3),
                                     perf_mode=DR)
                dst = v4[:, i, n * DC:(n + 1) * DC, 0:HS]
                nc.vector.tensor_add(dst,
                                     ps.rearrange("p (h d) -> p h d", d=HS),
                                     bvB[:, ts(n, 512)].rearrange(
                                         "p (h d) -> p h d", d=HS))

        # deferred work pulled between heads: qb0 interleaves remaining K/Q
        # projections; qb1 interleaves the previous block's FFN chunk jobs.
        if qb == 0:
            wv_t = pwv.tile([P, DC, D], FP8, name="wv_t")
            for c in range(DC):
                nc.sync.dma_start(out=wv_t[:, c, :], in_=io["wv"][:, c, :])
            for c in (0, 1):
                emit_kq_chunk(c, "k")
                emit_kq_chunk(c, "q")
            for i in range(TKC):
                emit_v_chunk(i)
            jobs = []
            for c in range(2, DC):
                jobs.append((emit_kq_chunk, (c, "k")))
                jobs.append((emit_kq_chunk, (c, "q")))
        else:
            jobs = [(_ffn1_job, (j, 0)) for j in range(FC)]
        job_i = 0

        exp_ctr = 0
        po_pair = None
        rcbs = {}
        for h in range(H):
            g, r0 = h // 2, 64 * (h % 2)
            local = h % 2
            p8 = pprob.tile([P, TKC, 512], FP8, tag="p", name=f"p8_{qb}_{h}")
            for kp in range(KP):
                ps_s = ppS.tile([P, 2, 512], F32, tag="s",
                                name=f"pss{qb}_{h}_{kp}")
                for i in range(2):
                    nc.tensor.matmul(
                        ps_s[:, i, :],
                        Kall[r0:r0 + 64, g:g + 2, ts(2 * kp + i, P)],
                        Q4[g][r0:r0 + 64, :, ts(qb, 512)],
                        perf_mode=DR)
                exp_ctr += 1
                if POOL_EXP_MOD < 1000 and exp_ctr % POOL_EXP_MOD == 1:
                    # GpSimd Schraudolph exp: int16 bits -> bf16 -> fp8
                    pi = pi16p.tile([P, 2, 512], mybir.dt.int16, tag="i",
                                    name=f"pi{qb}_{h}_{kp}")
                    nc.gpsimd.tensor_scalar(
                        out=pi, in0=ps_s, scalar1=schr_s, scalar2=schr_b,
                        op0=ALU.mult, op1=ALU.add)
                    nc.gpsimd.tensor_copy(out=p8[:, 2 * kp:2 * kp + 2, :],
                                          in_=pi.bitcast(BF16))
                else:
                    nc.scalar.activation(out=p8[:, 2 * kp:2 * kp + 2, :],
                                         in_=ps_s, func=AF.Exp,
                                         bias=negc_t, scale=0.125)
                if kp in (2, 5) and job_i < len(jobs):
                    fn, args = jobs[job_i]
                    fn(*args)
                    job_i += 1
            # PV: DoubleRow over key-chunk pairs; denom rides as ones col
            if local == 0:
                po_pair = ppOB.tile([P, 2, 512], F32, tag="ob",
                                    name=f"po{qb}_{g}")
            po = po_pair[0:HS + 1, local, :]
            for kp in range(KP):
                nc.tensor.matmul(
                    po, v_aug[:, 2 * kp:2 * kp + 2,
                              h * (HS + 1):(h + 1) * (HS + 1)],
                    p8[:, 2 * kp:2 * kp + 2, :],
                    start=(kp == 0), stop=(kp == KP - 1), perf_mode=DR)
            rc = prb.tile([1, 512], F32, tag="rc", name=f"rc{qb}_{h}")
            nc.vector.reciprocal(out=rc, in_=po_pair[HS:HS + 1, local, :])
            rcb = prb.tile([1, 512], BF16, tag="rcb", name=f"rcb{qb}_{h}")
            nc.vector.tensor_copy(out=rcb, in_=rc)
            rcbs[local] = rcb
            if local == 1:
                hp = h // 2
                rbp = ppRB.tile([P, 512], F32, tag="rb", name=f"rbp{qb}_{hp}")
                for lc in range(2):
                    nc.tensor.matmul(rbp[lc * HS:(lc + 1) * HS, :],
                                     onesK1[:, 0:HS], rcbs[lc])
                rbs = prb.tile([P, 512], BF16, tag="rbs", name=f"rbs{qb}_{hp}")
                nc.vector.tensor_copy(out=rbs, in_=rbp)
                for lc in range(2):
                    nc.vector.tensor_mul(
                        och8[lc * HS:(lc + 1) * HS, hp, ts(qb, 512)],
                        po_pair[0:HS, lc, :], rbs[lc * HS:(lc + 1) * HS, :])
            if job_i < len(jobs):
                fn, args = jobs[job_i]
                fn(*args)
                job_i += 1
        while job_i < len(jobs):
            fn, args = jobs[job_i]
            fn(*args)
            job_i += 1
        if qb == 1:
            for j in range(DC):
                _ffn2_job(j, 0)

        ppRB.release()
        ppOB.release()
        ppS.release()
        prb.release()
        pi16p.release()
        pprob.release()
        if qb == 0:
            pwv.release()
            pp_kqv.release()
            pwkv.release()
            pxln.release()
        else:
            ppFF.release()
            pfoc.release()
            pfb.release()
            pwf.release()

        # ---------------- proj + LN2 + gelu for this query block ---------
        pxqb = tc.alloc_tile_pool(name="pxqb", bufs=1)
        pwproj = tc.alloc_tile_pool(name="pwproj", bufs=8)
        pmid = tc.alloc_tile_pool(name="pmid", bufs=1)
        rowsM = tc.alloc_tile_pool(name="rowsM", bufs=4)
        pr2 = tc.alloc_tile_pool(name="pr2", bufs=1)
        ppE = tc.alloc_tile_pool(name="ppE", bufs=2, space="PSUM")
        ppM_st = tc.alloc_tile_pool(name="ppM_st", bufs=2, space="PSUM")
        ppM_b = tc.alloc_tile_pool(name="ppM_b", bufs=2, space="PSUM")

        xq_t = pxqb.tile([P, DC, 512], BF16, name=f"xq{qb}")
        for c in range(DC):
            nc.sync.dma_start(out=xq_t[:, c, :],
                              in_=xT_d[c][:, ts(qb, 512)])
        for j in range(DC):
            wj = pwproj.tile([P, 4, 2, P], FP8, tag="w", name=f"wpj{qb}_{j}")
            nc.sync.dma_start(out=wj, in_=io["wproj"][j])
            psn = ppE.tile([P, 512], F32, tag="mm", name=f"psp{qb}_{j}")
            for t in range(4):
                nc.tensor.matmul(psn, wj[:, t],
                                 och8[:, 2 * t:2 * t + 2, ts(qb, 512)],
                                 start=(t == 0), stop=(t == 3), perf_mode=DR)
            nc.vector.scalar_tensor_tensor(
                out=h_t[:, j, ts(qb, 512)], in0=psn,
                scalar=bproj_s[:, j:j + 1], in1=xq_t[:, j, :],
                op0=ALU.add, op1=ALU.add)

        hb = pmid.tile([P, DC, 512], BF16, tag="hb", name=f"hb{qb}")
        hsq = pmid.tile([P, DC, 512], BF16, tag="hs", name=f"hsq{qb}")
        for c in range(DC):
            nc.vector.tensor_copy(out=hb[:, c, :],
                                  in_=h_t[:, c, ts(qb, 512)])
            nc.vector.tensor_mul(hsq[:, c, :], hb[:, c, :], hb[:, c, :])
        r2B = pr2.tile([P, 512], BF16, tag="rb", name=f"r2B{qb}")
        mur2B = pr2.tile([P, 512], BF16, tag="mb", name=f"mur2B{qb}")
        ln_stats_bcast(ppM_st, ppM_b, rowsM, hb,
                       lambda c, kt: hsq[:, c, :], 512, r2B, mur2B,
                       f"2{qb}")
        gb = pmid.tile([P, DC, 512], BF16, tag="gb", name=f"gb{qb}")
        for c in range(DC):
            cen = pmid.tile([P, 512], BF16, tag="cen", name=f"cen{qb}_{c}")
            nc.vector.tensor_mul(cen, hb[:, c, :], r2B)
            nc.vector.tensor_sub(cen, cen, mur2B)
            nc.scalar.activation(out=gb[:, c, :], in_=cen, func=AF.Gelu,
                                 bias=bln2_s[:, c:c + 1],
                                 scale=g2_s[:, c:c + 1])
            nc.vector.tensor_copy(out=g8[:, c, :], in_=gb[:, c, :])
            if USE_GSPLIT:
                nc.vector.tensor_sub(gr8[:, c, :], gb[:, c, :], g8[:, c, :])
            if USE_W1LO:
                nc.gpsimd.tensor_scalar_mul(g64[:, c, :], g8[:, c, :],
                                            1.0 / LO_SCALE)

        ppM_b.release()
        ppM_st.release()
        ppE.release()
        pr2.release()
        rowsM.release()
        pmid.release()
        pwproj.release()
        pxqb.release()

        if qb == 0:
            # f1 pools live from here (ffn1 jobs inside the qb1 attention
            # group) until the tail FFN completes
            pf1 = tc.alloc_tile_pool(name="pf1", bufs=1)
            f1h = pf1.tile([P, FC, 512], FP8, name="f1h")
            f1r = pf1.tile([P, FC, 512], FP8, name="f1r")
            f1h64 = pf1.tile([P, FC, 512], FP8, name="f1h64")

            # bind the FFN chunk jobs for this block; they are pulled into
            # the next block's attention group (PE fills ACT-bound gaps)
            def _ffn1_job(j, _qb=qb):
                w1h_t = pwf.tile([P, 4, 2, P], FP8, tag="w1",
                                 name=f"w1h{_qb}_{j}")
                nc.sync.dma_start(out=w1h_t, in_=io["w1h"][j])
                if USE_W1LO:
                    w1l_t = pwf.tile([P, 4, 2, P], FP8, tag="w1",
                                     name=f"w1l{_qb}_{j}")
                    nc.sync.dma_start(out=w1l_t, in_=io["w1l"][j])
                psA = ppFF.tile([P, 512], F32, tag="mm", name=f"psf{_qb}_{j}")
                mms = [(w1h_t, g8)]
                if USE_GSPLIT:
                    mms.append((w1h_t, gr8))
                if USE_W1LO:
                    mms.append((w1l_t, g64))
                nmm = len(mms) * 4
                k = 0
                for wt, rhs in mms:
                    for t in range(4):
                        nc.tensor.matmul(psA, wt[:, t],
                                         rhs[:, 2 * t:2 * t + 2, :],
                                         start=(k == 0), stop=(k == nmm - 1),
                                         perf_mode=DR)
                        k += 1
                f1b = pfb.tile([P, 512], BF16, tag="fb", name=f"f1b{_qb}_{j}")
                nc.scalar.activation(out=f1b, in_=psA, func=AF.Gelu,
                                     bias=b1_s[:, j:j + 1], scale=1.0)
                nc.vector.tensor_copy(out=f1h[:, j, :], in_=f1b)
                if USE_F1SPLIT:
                    nc.vector.tensor_sub(f1r[:, j, :], f1b, f1h[:, j, :])
                if USE_W2LO:
                    nc.gpsimd.tensor_scalar_mul(f1h64[:, j, :], f1h[:, j, :],
                                                1.0 / LO_SCALE)

            def _ffn2_job(j, _qb=qb):
                w2h_t = pwf.tile([P, 16, 2, P], FP8, tag="w2",
                                 name=f"w2h{_qb}_{j}")
                nc.sync.dma_start(out=w2h_t, in_=io["w2h"][j])
                if USE_W2LO:
                    w2l_t = pwf.tile([P, 16, 2, P], FP8, tag="w2",
                                     name=f"w2l{_qb}_{j}")
                    nc.sync.dma_start(out=w2l_t, in_=io["w2l"][j])
                psA = ppFF.tile([P, 512], F32, tag="mm", name=f"pso{_qb}_{j}")
                mms = [(w2h_t, f1h)]
                if USE_F1SPLIT:
                    mms.append((w2h_t, f1r))
                if USE_W2LO:
                    mms.append((w2l_t, f1h64))
                nmm = len(mms) * 16
                k = 0
                for wt, rhs in mms:
                    for t in range(16):
                        nc.tensor.matmul(psA, wt[:, t],
                                         rhs[:, 2 * t:2 * t + 2, :],
                                         start=(k == 0), stop=(k == nmm - 1),
                                         perf_mode=DR)
                        k += 1
                outc = pfoc.tile([P, 512], F32, tag="oc",
                                 name=f"outc{_qb}_{j}")
                nc.vector.scalar_tensor_tensor(
                    out=outc, in0=psA, scalar=b2_s[:, j:j + 1],
                    in1=h_t[:, j, ts(_qb, 512)], op0=ALU.add, op1=ALU.add)
                nc.sync.dma_start(out=io["out"][j][:, ts(_qb, 512)],
                                  in_=outc)

    # ---------------- FFN for the last query block (tail) ----------------
    pwf = tc.alloc_tile_pool(name="pwfT", bufs=2)
    pfb = tc.alloc_tile_pool(name="pfbT", bufs=3)
    pfoc = tc.alloc_tile_pool(name="pfocT", bufs=2)
    ppFF = tc.alloc_tile_pool(name="ppFFT", bufs=2, space="PSUM")
    qb = 1

    def _f1j(j):
        w1h_t = pwf.tile([P, 4, 2, P], FP8, tag="w1", name=f"w1hT{j}")
        nc.sync.dma_start(out=w1h_t, in_=io["w1h"][j])
        if USE_W1LO:
            w1l_t = pwf.tile([P, 4, 2, P], FP8, tag="w1", name=f"w1lT{j}")
            nc.sync.dma_start(out=w1l_t, in_=io["w1l"][j])
        psA = ppFF.tile([P, 512], F32, tag="mm", name=f"psfT{j}")
        mms = [(w1h_t, g8)]
        if USE_GSPLIT:
            mms.append((w1h_t, gr8))
        if USE_W1LO:
            mms.append((w1l_t, g64))
        nmm = len(mms) * 4
        k = 0
        for wt, rhs in mms:
            for t in range(4):
                nc.tensor.matmul(psA, wt[:, t], rhs[:, 2 * t:2 * t + 2, :],
                                 start=(k == 0), stop=(k == nmm - 1),
                                 perf_mode=DR)
                k += 1
        f1b = pfb.tile([P, 512], BF16, tag="fb", name=f"f1bT{j}")
        nc.scalar.activation(out=f1b, in_=psA, func=AF.Gelu,
                             bias=b1_s[:, j:j + 1], scale=1.0)
        nc.vector.tensor_copy(out=f1h[:, j, :], in_=f1b)
        if USE_F1SPLIT:
            nc.vector.tensor_sub(f1r[:, j, :], f1b, f1h[:, j, :])
        if USE_W2LO:
            nc.gpsimd.tensor_scalar_mul(f1h64[:, j, :], f1h[:, j, :],
                                        1.0 / LO_SCALE)

    def _f2j(j):
        w2h_t = pwf.tile([P, 16, 2, P], FP8, tag="w2", name=f"w2hT{j}")
        nc.sync.dma_start(out=w2h_t, in_=io["w2h"][j])
        if USE_W2LO:
            w2l_t = pwf.tile([P, 16, 2, P], FP8, tag="w2", name=f"w2lT{j}")
            nc.sync.dma_start(out=w2l_t, in_=io["w2l"][j])
        psA = ppFF.tile([P, 512], F32, tag="mm", name=f"psoT{j}")
        mms = [(w2h_t, f1h)]
        if USE_F1SPLIT:
            mms.append((w2h_t, f1r))
        if USE_W2LO:
            mms.append((w2l_t, f1h64))
        nmm = len(mms) * 16
        k = 0
        for wt, rhs in mms:
            for t in range(16):
                nc.tensor.matmul(psA, wt[:, t], rhs[:, 2 * t:2 * t + 2, :],
                                 start=(k == 0), stop=(k == nmm - 1),
                                 perf_mode=DR)
                k += 1
        outc = pfoc.tile([P, 512], F32, tag="oc", name=f"outcT{j}")
        nc.vector.scalar_tensor_tensor(
            out=outc, in0=psA, scalar=b2_s[:, j:j + 1],
            in1=h_t[:, j, ts(qb, 512)], op0=ALU.add, op1=ALU.add)
        nc.sync.dma_start(out=io["out"][j][:, ts(qb, 512)], in_=outc)

    for j in range(FC):
        _f1j(j)
    for j in range(DC):
        _f2j(j)

    ppFF.release()
    pfoc.release()
    pfb.release()
    pwf.release()
    pf1.release()
    pvaug.release()
    pkq.release()
    pg.release()
    poch.release()
    ph.release()
    consts.release()

# ----------------------------------------------------------------------------
# host side
# ----------------------------------------------------------------------------

def _stripe(v):
    """[n*P] -> [P, n] per-partition striping (feature f = c*P + p)."""
    v = np.asarray(v, np.float32)
    return np.ascontiguousarray(v.reshape(-1, P).T)


def _dr_pack(W):
    """[Din, Dout] -> [Dout/P, P, Din/(2P), 2, P] fp8 DoubleRow lhsT tiles
    (partition dim first within each chunk so the DMA layout matches)."""
    din, dout = W.shape
    r = W.astype(e4m3).reshape(din // 256, 2, P, dout // P, P)
    return np.ascontiguousarray(r.transpose(3, 2, 0, 1, 4))


def prep_shared(inputs):
    f32 = np.float32
    g1 = np.asarray(inputs["ln1_g"], f32)
    b1n = np.asarray(inputs["ln1_b"], f32)
    W_ap = np.asarray(inputs["W_ap"], f32)
    b_ap = np.asarray(inputs["b_ap"], f32)
    W_qkv = np.asarray(inputs["W_qkv"], f32)
    b_qkv = np.asarray(inputs["b_qkv"], f32)
    W_proj = np.asarray(inputs["W_proj"], f32)
    W1 = np.asarray(inputs["W1"], f32)
    W2 = np.asarray(inputs["W2"], f32)

    # fold LN1 gamma and the attn pre-projection into W_qkv
    W_eff = (g1[:, None] * W_ap) @ W_qkv
    b_eff = (b_ap + b1n @ W_ap) @ W_qkv + b_qkv

    Wq = W_eff[:, 0:D]
    Wk = W_eff[:, D:2 * D]
    bq = b_eff[0:D]
    bk = b_eff[D:2 * D]

    W1h = W1.astype(e4m3).astype(f32)
    W1l = ((W1 - W1h) * LO_SCALE)
    W2h = W2.astype(e4m3).astype(f32)
    W2l = ((W2 - W2h) * LO_SCALE)

    shared = {
        "wq": _dr_pack(Wq),
        "wk": _dr_pack(Wk),
        "wv": np.ascontiguousarray(
            W_eff[:, 2 * D:].astype(e4m3).reshape(DC, P, D).transpose(1, 0, 2)),
        "bqk": np.concatenate([_stripe(bq), _stripe(bk)], axis=1),
        "bv": np.ascontiguousarray(np.asarray(b_eff[2 * D:], f32)),
        "wproj": _dr_pack(W_proj),
        "bproj": _stripe(np.asarray(inputs["b_proj"], f32)),
        "w1h": _dr_pack(W1h),
        "w1l": _dr_pack(W1l),
        "b1": _stripe(np.asarray(inputs["b1"], f32)),
        "w2h": _dr_pack(W2h),
        "w2l": _dr_pack(W2l),
        "b2": _stripe(np.asarray(inputs["b2"], f32)),
        "zeros": np.zeros(T, e4m3),
        "g2": _stripe(np.asarray(inputs["ln2_g"], f32)),
        "bln2": _stripe(np.asarray(inputs["ln2_b"], f32)),
    }
    return shared


def prep_core_x(x, core):
    b, qh = core // 2, core % 2
    xTb = np.asarray(x[b], np.float32).T  # [D, T] view
    if qh:
        xTb = np.concatenate([xTb[:, Tq:], xTb[:, :Tq]], axis=1)
    return np.ascontiguousarray(xTb.astype(bf16).reshape(DC, P, T))


def assemble_output(results, dtype):
    out = np.empty((B, T, D), dtype)
    for c in range(N_CORES):
        b, qh = c // 2, c % 2
        arr = np.asarray(results[c]["out"]).reshape(D, Tq)
        out[b, qh * Tq:(qh + 1) * Tq, :] = arr.T
    return out


def kernel(**inputs):
    x = np.asarray(inputs["x"], np.float32)
    shared = prep_shared(inputs)
    nc = build_nc()
    in_maps = [dict(shared, xt=prep_core_x(x, c)) for c in range(N_CORES)]
    res = run_bass_kernel_spmd(nc, in_maps, list(range(N_CORES)))
    return assemble_output(res.results, np.float32)


if __name__ == "__main__":
    nc = build_nc()
    print("built ok")
